# revision 27
# baseline (speedup 1.0000x reference)
"""Trainium2 Bass kernel for nn_LossMeanCov (softmax filling + argmin segment mean/cov loss).

Self-contained: hardcodes shapes N=131072, D=32, K=64, 8 cores.

Strategy (data-parallel over N, 16384 points/core):
  Kernel 1 (per core): distances g = cc - 2 x.c via one fp16 matmul per
    128-point tile ([points, K] layout); DVE segmented min -> m; DVE
    broadcast-subtract h = g - m (fp16); ACT exp -> E (bf16); DVE segmented
    sum -> s; gpsimd divide -> r; PE matmul with r as weights accumulates
    the soft-filling partial sums in PSUM (lagged behind the distance
    matmuls so the PE stream never stalls on the DVE/ACT chain); gpsimd
    is_equal(h, 0) emits the one-hot argmin matrix (uint8) for the host.
  Host: pred = argmax(one-hot); builds a cluster-sorted, 128-padded,
    tile-major fp16 layout of x (pure data movement).
  Kernel 2 (per core): whole sorted shard staged into SBUF with 4 large
    DMAs, then per-cluster second moments + sums as fp16 matmuls
    X'^T [X' | 1] into per-tile PSUM windows (4-way column-tiled).
  Host: sums partials over cores, forms means/covs, computes scalar loss.
"""

import sys
import numpy as np

sys.path.insert(0, "/opt/trn_rl_repo")

N, D, K = 131072, 32, 64
NCORES = 8
NLOC = N // NCORES          # 16384 points per core
NT = NLOC // 128            # 128 tiles of 128 points
BATCH = 16                  # tiles per processing batch (2 PSUM banks)
NB = NT // BATCH            # 8 batches
FILL_LAG = 2                # batches of lag before fill matmuls issue
SUM_LAG = True              # run sum/recip one batch behind min/sub on DVE
BETA = 10.0
KAPPA = 1.0

_CACHE = {}


def _bass_mods():
    import concourse.bacc as bacc
    import concourse.mybir as mybir
    from concourse.tile import TileContext
    from concourse.bass_utils import run_bass_kernel_spmd
    return bacc, mybir, TileContext, run_bass_kernel_spmd


def _build_k1(loop=1):
    bacc, mybir, TileContext, _ = _bass_mods()
    nc = bacc.Bacc("TRN2", target_bir_lowering=False)
    # rows 0..31: x^T (fp16), rows 32,33: ones (for the cc hi/lo pair)
    xt = nc.dram_tensor("xt", [34, NLOC], mybir.dt.float16, kind="ExternalInput")
    # rows 0..31: -2 c^T (fp16), row 32: cc_hi, row 33: cc_lo
    caug = nc.dram_tensor("caug", [34, K], mybir.dt.float16, kind="ExternalInput")
    a_out = nc.dram_tensor("a_out", [128, NT * K], mybir.dt.bfloat16, kind="ExternalOutput")
    # [BATCH, BATCH*K] cross-product matrix; host extracts diagonal blocks
    fill_out = nc.dram_tensor("fill_out", [BATCH, BATCH * K], mybir.dt.float32,
                              kind="ExternalOutput")

    with TileContext(nc) as tc:
        with tc.tile_pool(name="const", bufs=1) as constp, \
             tc.tile_pool(name="xtp", bufs=2) as xtp, \
             tc.tile_pool(name="gp", bufs=3, space="PSUM") as gp, \
             tc.tile_pool(name="fillp", bufs=1, space="PSUM") as fillp, \
             tc.tile_pool(name="hb", bufs=4) as hb, \
             tc.tile_pool(name="ab", bufs=2) as ab, \
             tc.tile_pool(name="mb", bufs=4) as mb, \
             tc.tile_pool(name="sb", bufs=4) as sbp, \
             tc.tile_pool(name="rb", bufs=FILL_LAG + 2) as rb:
            c_t = constp.tile([34, K], mybir.dt.float16)
            nc.sync.dma_start(out=c_t[:], in_=caug[:])
            fill_ps = fillp.tile([BATCH, BATCH * K], mybir.dt.float32)

            def one_pass(_i=None):
                # whole-shard xt staged in SBUF; 4 independent chunk tiles so
                # each 4-batch group only waits on its own DMA
                nq = NLOC // 4
                xt_q = []
                for q in range(4):
                    xq = xtp.tile([34, nq], mybir.dt.float16,
                                  tag=f"xt{q}", name=f"xt{q}")
                    nc.sync.dma_start(
                        out=xq[:], in_=xt[:, q * nq:(q + 1) * nq])
                    xt_q.append(xq)
                # e itself doubles as the argmin one-hot: h==0 -> e==1.0 is
                # the per-row max, so the host argmaxes the bf16 bit pattern.
                e_all = ab.tile([128, NT * K], mybir.dt.bfloat16,
                                tag="e_all", name="e_all")

                r_tiles = [None] * NB

                def fill_mms(b):
                    # r16 [128,16] x e-half [128,512] -> [16,512] of tile
                    # cross-products; only diagonal blocks are real, host
                    # discards the rest. 2 wide matmuls replace 16 tiny ones.
                    r_b = r_tiles[b]
                    for h in range(2):
                        nc.tensor.matmul(
                            fill_ps[:, h * 512:(h + 1) * 512],
                            lhsT=r_b[:],
                            rhs=e_all[:, b * BATCH * K + h * 512:
                                      b * BATCH * K + (h + 1) * 512],
                            start=(b == 0),
                            stop=(b == NB - 1),
                            skip_group_check=True)

                def sum_stage(b):
                    # runs one batch behind the min/sub/exp front so the DVE
                    # stream never waits on ACT's exp of the same batch
                    e_t = e_all[:, b * BATCH * K:(b + 1) * BATCH * K]
                    s_t = sbp.tile([128, BATCH], mybir.dt.float32, tag="s", name="s_t")
                    nc.vector.tensor_reduce(
                        s_t[:], e_t.rearrange("p (t k) -> p t k", k=K),
                        axis=mybir.AxisListType.X, op=mybir.AluOpType.add)
                    r16 = rb.tile([128, BATCH], mybir.dt.bfloat16, tag="r16", name="r16")
                    with nc.allow_low_precision("bf16 fill weights are enough"):
                        nc.vector.reciprocal(r16[:], s_t[:])
                    r_tiles[b] = r16
                    if ((b + 1) * BATCH) % (NT // 4) == 0:
                        # stream e out as each quarter completes
                        hi_ = (b + 1) * BATCH * K
                        lo = hi_ - (NT // 4) * K
                        nc.sync.dma_start(out=a_out[:, lo:hi_],
                                          in_=e_all[:, lo:hi_])

                for b in range(NB):
                    xq = xt_q[b * BATCH // (NT // 4)]
                    qoff = (b * BATCH % (NT // 4)) * 128
                    g_ps = gp.tile([128, BATCH * K], mybir.dt.float32,
                                   tag="g_ps", name="g_ps")
                    for t in range(BATCH):
                        nc.tensor.matmul(
                            g_ps[:, t * K:(t + 1) * K],
                            lhsT=xq[:, qoff + t * 128:qoff + (t + 1) * 128],
                            rhs=c_t[:],
                            start=True, stop=True)
                    # lagged fill matmuls keep the PE stream from stalling
                    # on the DVE/ACT chain of the current batch
                    if b >= FILL_LAG:
                        fill_mms(b - FILL_LAG)

                    g3 = g_ps[:].rearrange("p (t k) -> p t k", k=K)
                    m_t = mb.tile([128, BATCH], mybir.dt.float32, tag="m", name="m_t")
                    nc.vector.tensor_reduce(
                        m_t[:], g3, axis=mybir.AxisListType.X, op=mybir.AluOpType.min)
                    h_t = hb.tile([128, BATCH * K], mybir.dt.float16,
                                  tag="h_t", name="h_t")
                    mbb = m_t[:].unsqueeze(2).broadcast_to([128, BATCH, K])
                    nc.vector.tensor_tensor(
                        out=h_t[:].rearrange("p (t k) -> p t k", k=K),
                        in0=g3, in1=mbb, op=mybir.AluOpType.subtract)
                    e_t = e_all[:, b * BATCH * K:(b + 1) * BATCH * K]
                    nc.scalar.activation(
                        e_t, h_t[:], mybir.ActivationFunctionType.Exp, scale=-BETA)
                    if SUM_LAG and b >= 1:
                        sum_stage(b - 1)
                    elif not SUM_LAG:
                        sum_stage(b)
                if SUM_LAG:
                    sum_stage(NB - 1)
                for b in range(NB - FILL_LAG, NB):
                    fill_mms(b)

            if loop == 1:
                one_pass()
            else:
                with tc.For_i(0, loop, 1) as i:
                    one_pass(i)

            fill_sb = mb.tile([BATCH, BATCH * K], mybir.dt.float32, tag="fill")
            nc.scalar.copy(fill_sb[:], fill_ps[:])
            nc.sync.dma_start(out=fill_out[:], in_=fill_sb[:])
    nc.compile()
    return nc


def _build_k2(caps, loop=1):
    """caps: tuple of 64 ints (multiples of 128) — per-cluster row capacity."""
    bacc, mybir, TileContext, _ = _bass_mods()
    ntiles = [c // 128 for c in caps]
    total_tiles = sum(ntiles)
    nc = bacc.Bacc("TRN2", target_bir_lowering=False)
    # tile-major sorted/padded points: [128, total_tiles, 33] fp16
    # col 32 is 1.0 for real rows, 0.0 for padding.
    fw = -(-total_tiles // 32)          # free windows per (bank, strip)
    assert fw * 33 <= 512
    xs = nc.dram_tensor("xs", [128, total_tiles, 33], mybir.dt.float16,
                        kind="ExternalInput")
    mom = nc.dram_tensor("mom", [8, 128, fw * 33], mybir.dt.float32,
                         kind="ExternalOutput")

    # chunk boundaries (in tiles) for the 4 staging DMAs
    nchunk = 4
    bounds = [round(q * total_tiles / nchunk) for q in range(nchunk + 1)]

    with TileContext(nc) as tc:
        with tc.tile_pool(name="xsp", bufs=2) as xsp, \
             tc.tile_pool(name="accp", bufs=1, space="PSUM") as accp, \
             tc.tile_pool(name="outp", bufs=2) as outp:
            acc = [accp.tile([128, fw * 33], mybir.dt.float32,
                             tag=f"acc{i}", name=f"acc{i}") for i in range(8)]

            def body(_i=None):
                chunks = []
                for q in range(nchunk):
                    t0, t1 = bounds[q], bounds[q + 1]
                    xq = xsp.tile([128, (t1 - t0) * 33], mybir.dt.float16,
                                  tag=f"xq{q}", name=f"xq{q}")
                    nc.sync.dma_start(out=xq[:], in_=xs[:, t0:t1, :])
                    chunks.append(xq)
                w = 0
                for q in range(nchunk):
                    xq = chunks[q]
                    for j in range(bounds[q + 1] - bounds[q]):
                        strip = w % 4
                        bank = (w // 4) % 8
                        f = w // 32
                        nc.tensor.matmul(
                            acc[bank][32 * strip:32 * (strip + 1),
                                      33 * f:33 * f + 33],
                            lhsT=xq[:, j * 33:j * 33 + 32],
                            rhs=xq[:, j * 33:(j + 1) * 33],
                            start=True, stop=True,
                            tile_position=(0, 32 * strip))
                        w += 1
                # copy the 8 PSUM banks to SBUF (split ACT/DVE), 1 DMA out
                ob = outp.tile([128, 8 * fw * 33], mybir.dt.float32,
                               tag="ob", name="ob")
                for i in range(8):
                    dst = ob[:, i * fw * 33:(i + 1) * fw * 33]
                    if i % 2 == 0:
                        nc.scalar.copy(dst, acc[i][:])
                    else:
                        nc.vector.tensor_copy(dst, acc[i][:])
                nc.sync.dma_start(
                    out=mom[:].rearrange("b p f -> p b f"),
                    in_=ob[:].rearrange("p (b f) -> p b f", b=8))

            if loop == 1:
                body()
            else:
                with tc.For_i(0, loop, 1) as i:
                    body(i)
    nc.compile()
    return nc


def _get_k1():
    if "k1" not in _CACHE:
        _CACHE["k1"] = _build_k1()
    return _CACHE["k1"]


def _get_k2(caps):
    key = ("k2", caps)
    if key not in _CACHE:
        _CACHE[key] = _build_k2(caps)
    return _CACHE[key]


def _run(nc, in_maps, trace=False):
    *_, run_bass_kernel_spmd = _bass_mods()
    return run_bass_kernel_spmd(nc, in_maps, core_ids=list(range(NCORES)),
                                trace=trace)


_LAST_TIMES = {}


def _prep_k1_inputs(x, c):
    cc = (c * c).sum(1)                       # [K]
    cch = cc.astype(np.float16)
    ccl = (cc - cch.astype(np.float32)).astype(np.float16)
    caug = np.concatenate(
        [(-2.0 * c.T).astype(np.float16), cch[None, :], ccl[None, :]], axis=0)
    shards = x.reshape(NCORES, NLOC, D)
    ones2 = np.ones((2, NLOC), dtype=np.float16)
    in_maps = []
    for s in range(NCORES):
        xt = np.concatenate([shards[s].T.astype(np.float16), ones2], axis=0)
        in_maps.append({"xt": np.ascontiguousarray(xt), "caug": caug})
    return in_maps, shards


def _preds_from_k1(r1):
    preds = np.empty((NCORES, NLOC), dtype=np.int64)
    for s in range(NCORES):
        # a_out holds e = exp(-beta*(g - min)) in bf16; its row max (1.0 at
        # the argmin, where h == 0 exactly) identifies pred. bf16 >= 0, so
        # the uint16 bit pattern is order-preserving.
        A = np.ascontiguousarray(r1.results[s]["a_out"]).view(np.uint16)
        preds[s] = A.reshape(128, NT, K).argmax(axis=2).T.reshape(NLOC)
    return preds


def _prep_k2_inputs(shards, preds, counts_pc, caps):
    ntiles = [cp // 128 for cp in caps]
    total_tiles = sum(ntiles)
    offs = np.concatenate([[0], np.cumsum(caps)])[:K]
    in_maps = []
    for s in range(NCORES):
        xs = np.zeros((total_tiles * 128, 33), dtype=np.float16)
        pred = preds[s]
        order = np.argsort(pred, kind="stable")
        sorted_pred = pred[order]
        starts = np.concatenate([[0], np.cumsum(counts_pc[s])])[:K]
        within = np.arange(NLOC) - starts[sorted_pred]
        dest = offs[sorted_pred] + within
        xs[dest, :D] = shards[s][order].astype(np.float16)
        xs[dest, D] = 1.0
        xs_pm = np.ascontiguousarray(
            xs.reshape(total_tiles, 128, 33).transpose(1, 0, 2))
        in_maps.append({"xs": xs_pm})
    return in_maps, ntiles, total_tiles


def kernel(x, cluster_centers, filling_target, means_target, covs_target,
           _trace=False):
    x = np.asarray(x, dtype=np.float32)
    c = np.asarray(cluster_centers, dtype=np.float32)
    filling_target = np.asarray(filling_target, dtype=np.float32)
    means_target = np.asarray(means_target, dtype=np.float32)
    covs_target = np.asarray(covs_target, dtype=np.float32)

    in_maps1, shards = _prep_k1_inputs(x, c)
    r1 = _run(_get_k1(), in_maps1, trace=_trace)
    _LAST_TIMES["k1"] = r1.exec_time_ns

    # ---- host: pred, counts, fill ----
    fill_sum = np.zeros(K, dtype=np.float64)
    for s in range(NCORES):
        F = r1.results[s]["fill_out"].astype(np.float64)
        F = F.reshape(BATCH, 2, BATCH // 2, K)
        for t in range(BATCH):
            fill_sum += F[t, t // (BATCH // 2), t % (BATCH // 2)]
    filling = (fill_sum / N).astype(np.float32)
    loss_fil = np.mean((filling - filling_target) ** 2)

    preds = _preds_from_k1(r1)
    counts_pc = np.zeros((NCORES, K), dtype=np.int64)
    for s in range(NCORES):
        counts_pc[s] = np.bincount(preds[s], minlength=K)
    counts = counts_pc.sum(0)

    caps = tuple(int(max(1, -(-int(counts_pc[:, k].max()) // 128)) * 128)
                 for k in range(K))

    in_maps2, ntiles, total_tiles = _prep_k2_inputs(shards, preds, counts_pc, caps)
    r2 = _run(_get_k2(caps), in_maps2, trace=_trace)
    _LAST_TIMES["k2"] = r2.exec_time_ns

    # ---- host: combine moments, compute loss ----
    fw = -(-total_tiles // 32)
    m2 = np.zeros((K, D, D), dtype=np.float64)
    sums = np.zeros((K, D), dtype=np.float64)
    tile_cluster = np.repeat(np.arange(K), ntiles)
    for s in range(NCORES):
        mom = r2.results[s]["mom"]            # [8, 128, fw*33]
        for w in range(total_tiles):
            k = tile_cluster[w]
            strip = w % 4
            bank = (w // 4) % 8
            f = w // 32
            W = mom[bank][32 * strip:32 * (strip + 1), 33 * f:33 * f + 33]
            m2[k] += W[:, :D]
            sums[k] += W[:, D]

    denom = np.maximum(counts.astype(np.float64), 1.0)
    means = sums / denom[:, None]
    covs = m2 / denom[:, None, None] - means[:, :, None] * means[:, None, :]
    loss_stat = np.mean((means - means_target.astype(np.float64)) ** 2) \
        + np.mean((covs - covs_target.astype(np.float64)) ** 2)
    total = loss_fil + KAPPA * loss_stat
    return np.float32(total)
